# revision 37
# baseline (speedup 1.0000x reference)
"""Cross-attention (B=4, Sq=4096, Sk=1024, H=16, D=1024) on 8 TRN2 NeuronCores.

Sharding: tensor-parallel by heads. Core c owns heads (2c, 2c+1), i.e. columns
[128c, 128c+128) of Wq/Wk/Wv and rows [128c, 128c+128) of Wo.

v5 design notes (vs v2 baseline at ~476us):
  - q-proj runs fp8e4 DoubleRow (two 128-contraction chunks fused per MM,
    ~1.7x on that stream); wq is host-scaled x64 into fp8's normal range and
    the inverse scale folds into the existing qt evac multiply. Everything
    on the value path (k/v-proj, att@v, out-proj, A2A payload) stays bf16:
    each fp8e4 quantization of a random-sign operand costs ~2.5% rel error
    (signal and noise both scale as sqrt(N)), which a 2e-2 gate cannot
    absorb (full-fp8 v4 measured 4.4e-2). fp8 xt also halves x DMA traffic.
  - Startup reordered: minimal prefix (q(0,0), k(0,*), v(0,0..1)) before the
    first scores, everything else rides as fillers; first exp ~10us vs 38us.
  - DMA queues split: sync = bulk input loads + rv gathers + out stores;
    gpsimd = the self-contained normalization chain (rb/bc rides, norm
    multiplies, sends, collectives). In v2 a single in-order sync queue
    head-blocked latency-critical sends behind 2MB xt loads, starving ACT
    ~10us at every batch boundary; in v4 the rv gather's wait-on-A2A head-
    blocked the gpsimd queue for ~14us per boundary.
  - Out-proj chunks ride inside attention slots of later batches instead of
    a ~50us post-attention drain; only batch 3's out-proj (+A2A(3) wait)
    remains in the tail.

Host prep: x pre-transposed/chunked to fp8e4, y/W* to bf16; scores matmuls
bf16; all PSUM accumulation fp32; output fp32.
"""

import numpy as np
import ml_dtypes

import concourse.bass as bass
import concourse.mybir as mybir
from concourse import bacc, tile
from concourse import bass_utils

BF16 = mybir.dt.bfloat16
F32 = mybir.dt.float32
F8E4 = mybir.dt.float8e4

B = 4
SQ = 4096
SK = 1024
D = 1024
DC = 768
NCORES = 8
SQL = SQ // NCORES  # 512 output rows per batch per core
KC = D // 128       # 8 contraction chunks for q-proj / out-proj
FC = DC // 128      # 6 contraction chunks for k/v-proj
JC = SK // 128      # 8 key chunks
NI = SQ // 512      # 8 query blocks of 512 per batch

Exp = mybir.ActivationFunctionType.Exp
Alu = mybir.AluOpType
DR = mybir.MatmulPerfMode.DoubleRow


def build_nc():
    nc = bacc.Bacc(
        "TRN2",
        target_bir_lowering=False,
        debug=False,
        num_devices=NCORES,
    )

    xt = nc.dram_tensor("xt", [B, KC, 128, SQ], F8E4, kind="ExternalInput")
    yt = nc.dram_tensor("yt", [B, FC, 128, SK], BF16, kind="ExternalInput")
    wq = nc.dram_tensor("wq", [KC, 128, 128], F8E4, kind="ExternalInput")
    wk = nc.dram_tensor("wk", [FC, 128, 128], BF16, kind="ExternalInput")
    wv = nc.dram_tensor("wv", [FC, 128, 128], BF16, kind="ExternalInput")
    wo = nc.dram_tensor("wo", [KC, 128, D], BF16, kind="ExternalInput")
    bq = nc.dram_tensor("bq", [128, 1], F32, kind="ExternalInput")
    bk = nc.dram_tensor("bk", [128, 1], F32, kind="ExternalInput")
    bvb = nc.dram_tensor("bvb", [128, 128], F32, kind="ExternalInput")
    bob = nc.dram_tensor("bob", [128, D], F32, kind="ExternalInput")
    out = nc.dram_tensor("out", [B, SQL, D], F32, kind="ExternalOutput")

    # DRAM bounce buffers for the per-batch AllToAll. Per dest core:
    # rows 0:64 = head A vals, 64:128 = head B vals (already normalized).
    # The last batch's A2A is split into two query-half collectives so its
    # out-proj can overlap the second half's wire time; collectives need
    # contiguous buffers, so the halves are separate tensors.
    send = [
        nc.dram_tensor(f"a2a_send_{b}", [NCORES, 128, 512], BF16, kind="Internal")
        for b in range(B - 1)
    ] + [[
        nc.dram_tensor(f"a2a_send3_{hf}", [NCORES, 128, 256], BF16, kind="Internal")
        for hf in range(2)
    ]]
    recv = [
        nc.dram_tensor(f"a2a_recv_{b}", [NCORES, 128, 512], BF16, kind="Internal")
        for b in range(B - 1)
    ] + [[
        nc.dram_tensor(f"a2a_recv3_{hf}", [NCORES, 128, 256], BF16, kind="Internal")
        for hf in range(2)
    ]]

    with tile.TileContext(nc) as tc:
        _program(nc, tc, xt, yt, wq, wk, wv, wo, bq, bk, bvb, bob, out, send, recv)
    nc.finalize()
    return nc


def _program(nc, tc, xt, yt, wq, wk, wv, wo, bq, bk, bvb, bob, out, send, recv):
    from contextlib import ExitStack

    with ExitStack() as ctx:
        const = ctx.enter_context(tc.tile_pool(name="const", bufs=1))
        ytp = ctx.enter_context(tc.tile_pool(name="ytp", bufs=2))
        xtp = ctx.enter_context(tc.tile_pool(name="xtp", bufs=10))
        qtp = ctx.enter_context(tc.tile_pool(name="qtp", bufs=2))
        ktp = ctx.enter_context(tc.tile_pool(name="ktp", bufs=2))
        vtp = ctx.enter_context(tc.tile_pool(name="vtp", bufs=16))
        ep = ctx.enter_context(tc.tile_pool(name="ep", bufs=3))
        attp = ctx.enter_context(tc.tile_pool(name="attp", bufs=6))
        attup = ctx.enter_context(tc.tile_pool(name="attup", bufs=10))
        recp = ctx.enter_context(tc.tile_pool(name="recp", bufs=4))
        recbp = ctx.enter_context(tc.tile_pool(name="recbp", bufs=8))
        bcp = ctx.enter_context(tc.tile_pool(name="bcp", bufs=10))
        rvp = ctx.enter_context(tc.tile_pool(name="rvp", bufs=3))
        outp = ctx.enter_context(tc.tile_pool(name="outp", bufs=3))
        rbp = ctx.enter_context(tc.tile_pool(name="rbp", bufs=8, space="DRAM"))
        # PSUM: scores 2x2 banks + nout 2x1 + proj 2x1 = 8 banks
        scp = ctx.enter_context(tc.tile_pool(name="scp", bufs=2, space="PSUM"))
        noutp = ctx.enter_context(tc.tile_pool(name="noutp", bufs=2, space="PSUM"))
        projp = ctx.enter_context(tc.tile_pool(name="projp", bufs=2, space="PSUM"))

        # ---- ACT warmup: pull the exp table load off the critical path
        warm = const.tile([128, 1], F32, tag="warm")
        nc.vector.memset(warm[:, :], 0.0)
        warm_o = const.tile([128, 1], F32, tag="warmo")
        nc.scalar.activation(warm_o[:, :], warm[:, :], Exp)

        # ---- constants / weights resident in SBUF
        bq_sb = const.tile([128, 1], F32, tag="bq")
        nc.sync.dma_start(out=bq_sb[:, :], in_=bq[:, :])
        bk_sb = const.tile([128, 1], F32, tag="bk")
        nc.sync.dma_start(out=bk_sb[:, :], in_=bk[:, :])
        bvb_sb = const.tile([128, 128], F32, tag="bvb")
        nc.sync.dma_start(out=bvb_sb[:, :], in_=bvb[:, :])

        wq_sb = const.tile([128, KC, 128], F8E4, tag="wq")

        def emit_wq_load():
            nc.sync.dma_start(
                out=wq_sb[:, :, :],
                in_=wq[:, :, :].rearrange("k p c -> p k c"),
            )
        wk_sb = const.tile([128, FC, 128], BF16, tag="wk")
        wv_sb = const.tile([128, FC, 128], BF16, tag="wv")

        def emit_wkv_load():
            nc.sync.dma_start(
                out=wk_sb[:, :, :],
                in_=wk[:, :, :].rearrange("f p c -> p f c"),
            )
            nc.sync.dma_start(
                out=wv_sb[:, :, :],
                in_=wv[:, :, :].rearrange("f p c -> p f c"),
            )
        wo_sb = const.tile([128, KC, D], BF16, tag="wo")
        bob_sb = const.tile([128, D], F32, tag="bob")

        def emit_wo_load():
            nc.sync.dma_start(
                out=wo_sb[:, :, :],
                in_=wo[:, :, :].rearrange("k p c -> p k c"),
            )
            nc.sync.dma_start(out=bob_sb[:, :], in_=bob[:, :])

        yt_d = {}
        kt_d = {}
        qt_d = {}
        xt_d = {}
        v_tiles = {}
        att_d = {}
        rvs_d = {}
        o_d = {}

        def emit_yt_load(pb):
            t = ytp.tile([128, FC, SK], BF16, name=f"yt_{pb}", tag="yt")
            nc.sync.dma_start(
                out=t[:, :, :],
                in_=yt[pb, :, :, :].rearrange("f p c -> p f c"),
            )
            yt_d[pb] = t
            kt_d[pb] = ktp.tile([128, SK], BF16, name=f"kt_{pb}", tag="kt")
            qt_d[pb] = qtp.tile([128, SQ], BF16, name=f"qt_{pb}", tag="qt")

        def emit_xt_load(pb, i5):
            t = xtp.tile([128, KC, 512], F8E4, name=f"xt_{pb}_{i5}", tag="xt")
            nc.sync.dma_start(
                out=t[:, :, :],
                in_=xt[pb, :, :, i5 * 512:(i5 + 1) * 512].rearrange("k p c -> p k c"),
            )
            xt_d[(pb, i5)] = t

        def emit_xt_slot(slot):
            # absolute q-block slot -> (batch, i5)
            if slot < B * NI:
                emit_xt_load(slot // NI, slot % NI)

        def emit_k_chain(pb, j2):
            yt_sb = yt_d[pb]
            kps = projp.tile([128, 512], F32, name=f"kps_{pb}_{j2}", tag="proj")
            for fc in range(FC):
                nc.tensor.matmul(
                    kps[:, :],
                    lhsT=wk_sb[:, fc, :],
                    rhs=yt_sb[:, fc, j2 * 512:(j2 + 1) * 512],
                    start=(fc == 0),
                    stop=(fc == FC - 1),
                )
            nc.vector.tensor_scalar_add(
                kt_d[pb][:, j2 * 512:(j2 + 1) * 512], kps[:, :], bk_sb[:, :]
            )

        def emit_v_chain(pb, jc):
            # v_aug layout per tile [128, 130]:
            #   cols 0:64  = head-A values, col 64  = ones (A sums)
            #   cols 65:129 = head-B values, col 129 = ones (B sums)
            yt_sb = yt_d[pb]
            vps = projp.tile([128, 128], F32, name=f"vps_{pb}_{jc}", tag="proj")
            for fc in range(FC):
                nc.tensor.matmul(
                    vps[:, :],
                    lhsT=yt_sb[:, fc, jc * 128:(jc + 1) * 128],
                    rhs=wv_sb[:, fc, :],
                    start=(fc == 0),
                    stop=(fc == FC - 1),
                )
            v_t = vtp.tile([128, 130], BF16, name=f"v_{pb}_{jc}", tag="vt")
            nc.vector.tensor_tensor(
                out=v_t[:, 0:130].rearrange("p (h x) -> p h x", h=2)[:, :, 0:64],
                in0=vps[:, :].rearrange("p (h x) -> p h x", h=2),
                in1=bvb_sb[:, :].rearrange("p (h x) -> p h x", h=2),
                op=Alu.add,
            )
            nc.vector.memset(v_t[:, 64:65], 1.0)
            nc.vector.memset(v_t[:, 129:130], 1.0)
            v_tiles[(pb, jc)] = v_t

        def emit_q_chain(pb, i5):
            xt_sb = xt_d.pop((pb, i5))
            qps = projp.tile([128, 512], F32, name=f"qps_{pb}_{i5}", tag="proj")
            for kp in range(KC // 2):
                nc.tensor.matmul(
                    qps[:, :],
                    lhsT=wq_sb[:, 2 * kp:2 * kp + 2, :],
                    rhs=xt_sb[:, 2 * kp:2 * kp + 2, :],
                    start=(kp == 0),
                    stop=(kp == KC // 2 - 1),
                    perf_mode=DR,
                )
            # wq is host-scaled x64 into fp8's normal range; bq is pre-scaled
            # x64 on the host so (psum + bq*64) * (0.125/64) = (q + bq) / 8
            nc.vector.tensor_scalar(
                out=qt_d[pb][:, i5 * 512:(i5 + 1) * 512],
                in0=qps[:, :],
                scalar1=bq_sb[:, :],
                scalar2=0.125 / 64.0,
                op0=Alu.add,
                op1=Alu.mult,
            )

        def emit_rv_gather(ob, hf=None):
            # Emitted on the gpsimd queue right AFTER the collective, so the
            # trigger never waits: an A2A-gated gather on an in-order queue
            # head-blocks everything behind it (40us stall in v5).
            if ob not in rvs_d:
                rvs_d[ob] = rvp.tile([128, KC, 512], BF16, name=f"rv_{ob}", tag="rv")
            if hf is None:
                nc.gpsimd.dma_start(
                    out=rvs_d[ob][:, :, :],
                    in_=recv[ob][:, :, :].rearrange("k p c -> p k c"),
                )
            else:
                nc.gpsimd.dma_start(
                    out=rvs_d[ob][:, :, hf * 256:(hf + 1) * 256],
                    in_=recv[ob][hf][:, :, :].rearrange("k p c -> p k c"),
                )

        def emit_outproj_chunk(ob, chunk):
            i1, eh = divmod(chunk, 2)
            rvs = rvs_d[ob]
            ops = projp.tile([128, 512], F32, name=f"ops_{ob}_{chunk}", tag="proj")
            for cc in range(KC):
                nc.tensor.matmul(
                    ops[:, :],
                    lhsT=rvs[:, cc, i1 * 128:(i1 + 1) * 128],
                    rhs=wo_sb[:, cc, eh * 512:(eh + 1) * 512],
                    start=(cc == 0),
                    stop=(cc == KC - 1),
                )
            if eh == 0:
                o_d[(ob, i1)] = outp.tile(
                    [128, 1024], F32, name=f"o_{ob}_{i1}", tag="o"
                )
            o_t = o_d[(ob, i1)]
            nc.vector.tensor_add(
                o_t[:, eh * 512:(eh + 1) * 512], ops[:, :],
                bob_sb[:, eh * 512:(eh + 1) * 512],
            )
            if eh == 1:
                nc.sync.dma_start(
                    out=out[ob, i1 * 128:(i1 + 1) * 128, :], in_=o_t[:, :]
                )

        # ---- startup: minimal prefix so the first exp lands ~10us in. The
        # rest of batch 0's prep rides as fillers inside its attention loop.
        emit_wq_load()
        emit_xt_slot(0)
        emit_yt_load(0)
        emit_wkv_load()
        emit_xt_slot(1)
        emit_xt_slot(2)
        emit_q_chain(0, 0)
        emit_k_chain(0, 0)
        emit_k_chain(0, 1)
        for jc in range(6):
            emit_v_chain(0, jc)

        # ---- filler schedule, keyed by absolute slot (b*8+i5).
        # q(0,t) at slot t-1; q(1,t) shifted +2 (slots 2..9); q(b>=2,t) at
        # slot (b-1)*8+t. v(0,2..7) inside slot 0; v(1,t) shifted like q.
        from collections import defaultdict
        fills = defaultdict(list)

        for t in range(1, NI):
            fills[t - 1].append(lambda j=t: emit_q_chain(0, j))
        for jc in range(6, JC):
            fills[0].append(lambda j=jc: emit_v_chain(0, j))
        fills[1].append(emit_wo_load)
        for t in range(NI):
            fills[t + 2].append(lambda j=t: emit_q_chain(1, j))
            fills[t].append(lambda j=t: emit_v_chain(1, j))
        fills[2].append(lambda: emit_k_chain(1, 0))
        fills[3].append(lambda: emit_k_chain(1, 1))
        for b in range(2, B):
            for t in range(NI):
                fills[(b - 1) * 8 + t].append(lambda pb=b, j=t: emit_q_chain(pb, j))
                fills[(b - 1) * 8 + t].append(lambda pb=b, j=t: emit_v_chain(pb, j))
            fills[(b - 1) * 8 + 0].append(lambda pb=b: emit_k_chain(pb, 0))
            fills[(b - 1) * 8 + 1].append(lambda pb=b: emit_k_chain(pb, 1))
        # yt(b) loads: yt(0), yt(1) at startup; yt(2) at slot 6; yt(3) at 14
        emit_yt_load(1)
        fills[6].append(lambda: emit_yt_load(2))
        fills[14].append(lambda: emit_yt_load(3))
        # xt prefetch, 2-slot lead over consumption: blocks 0-2 at startup;
        # b0 blocks 3-7 consumed at slot s-1; b1 blocks 8-15 consumed at
        # slot s-6 (q(1,t) shifted +2); b>=2 blocks consumed at slot s-8
        for s in range(3, 8):
            fills[s - 3].append(lambda ss=s: emit_xt_slot(ss))
        for s in range(8, 16):
            fills[s - 8].append(lambda ss=s: emit_xt_slot(ss))
        for T in range(6, B * NI):
            fills[T].append(lambda ss=T + 10: emit_xt_slot(ss))

        # out-proj chunk placement: the tile scheduler hoists out-proj
        # LDWEIGHTS (whose only dependency is the gathered rv tile) several
        # slots ahead in the in-order tensor queue. A chunk emitted earlier
        # than ~2 batches after its A2A gets hoisted to before the A2A even
        # fires and head-blocks the queue ~40us. So outproj(0) rides in
        # batch 3 and outproj(1,2) in the drain, where their rv tiles are
        # long since ready no matter how far the LDW is hoisted.
        OP_SCHED = {(3, i): [(0, i)] for i in range(8)}
        DRAIN_OP = [(ob, ci) for ob in (1, 2) for ci in range(8)]

        pend_norm = []
        e_carry = None

        for b in range(B):
            kt_sb = kt_d[b]
            qt_sb = qt_d[b]

            for i5 in range(NI):
                slot = b * NI + i5
                fill = list(fills.pop(slot, ()))
                for ob, cc in OP_SCHED.get((b, i5), []):
                    fill.append(lambda ob=ob, cc=cc: emit_outproj_chunk(ob, cc))

                isl = slice(i5 * 512, (i5 + 1) * 512)
                na = noutp.tile([65, 512], F32, name=f"na_{b}_{i5}", tag="nout")
                nb = noutp.tile([65, 512], F32, name=f"nb_{b}_{i5}", tag="nout")

                def emit_scores(sb, si5, jc):
                    sc = scp.tile([128, 1024], F32, name=f"sc_{sb}_{si5}_{jc}", tag="sc")
                    jsl = slice(jc * 128, (jc + 1) * 128)
                    sisl = slice(si5 * 512, (si5 + 1) * 512)
                    skt = kt_d[sb]
                    sqt = qt_d[sb]
                    # scoresT for both heads, row-tiled (K=64 each, concurrent)
                    nc.tensor.matmul(
                        sc[:, 0:512],
                        lhsT=skt[0:64, jsl],
                        rhs=sqt[0:64, sisl],
                        start=True, stop=True,
                    )
                    nc.tensor.matmul(
                        sc[:, 512:1024],
                        lhsT=skt[64:128, jsl],
                        rhs=sqt[64:128, sisl],
                        start=True, stop=True,
                    )
                    e_t = ep.tile([128, 1024], BF16, name=f"e_{sb}_{si5}_{jc}", tag="e")
                    nc.scalar.activation(e_t[:, :], sc[:, :], Exp)
                    return e_t

                # software-pipelined over jc AND across i5: scores for the
                # next block's jc=0 are emitted before this block's last av,
                # so ACT never drains at i5 boundaries. e_carry holds the
                # exp tile for (b, i5, jc=0) produced by the previous block.
                if e_carry is None:
                    e_carry = emit_scores(b, i5, 0)
                e_cur = e_carry
                nfill = len(fill)
                for jc in range(JC):
                    f0 = jc * nfill // JC
                    f1 = (jc + 1) * nfill // JC
                    for f in fill[f0:f1]:
                        f()
                    if jc + 1 < JC:
                        e_next = emit_scores(b, i5, jc + 1)
                    elif i5 + 1 < NI:
                        e_next = emit_scores(b, i5 + 1, 0)
                    elif b + 1 < B:
                        e_next = emit_scores(b + 1, 0, 0)
                    else:
                        e_next = None
                    v_t = v_tiles[(b, jc)]
                    nc.tensor.matmul(
                        na[:, :],
                        lhsT=v_t[:, 0:65],
                        rhs=e_cur[:, 0:512],
                        start=(jc == 0),
                        stop=(jc == JC - 1),
                    )
                    nc.tensor.matmul(
                        nb[:, :],
                        lhsT=v_t[:, 65:130],
                        rhs=e_cur[:, 512:1024],
                        start=(jc == 0),
                        stop=(jc == JC - 1),
                    )
                    e_cur = e_next
                e_carry = e_cur

                # evacuate nout psum, normalize by the sums row, stage fp8 att
                # tiles; one send DMA per (i5-pair, head)
                if i5 % 2 == 0:
                    att_d[0] = attp.tile([64, 1024], BF16, name=f"attA_{b}_{i5}", tag="att")
                    att_d[1] = attp.tile([64, 1024], BF16, name=f"attB_{b}_{i5}", tag="att")
                hsl = slice((i5 % 2) * 512, (i5 % 2) * 512 + 512)
                # emit the PREVIOUS i5's deferred broadcast+multiply first:
                # by now its rb ride has landed, so the gpsimd queue never
                # stalls on the DRAM round-trip
                for fn in pend_norm:
                    fn()
                pend_norm = []
                for h, nres in ((0, na), (1, nb)):
                    att_u = attup.tile([65, 512], BF16, name=f"au_{b}_{i5}_{h}", tag="au")
                    nc.vector.tensor_copy(att_u[:, :], nres[:, :])
                    rec = recp.tile([65, 512], F32, name=f"rec_{b}_{i5}_{h}", tag="rec")
                    nc.vector.reciprocal_approx_fast(out=rec[:, :], in_=nres[:, :])
                    recb = recbp.tile([65, 512], BF16, name=f"rcb_{b}_{i5}_{h}", tag="rcb")
                    nc.vector.tensor_copy(recb[:, :], rec[:, :])
                    rb = rbp.tile([1, 512], BF16, name=f"rb_{b}_{i5}_{h}", tag="rb")
                    nc.gpsimd.dma_start(out=rb[:, :], in_=recb[64:65, :])

                    def norm_tail(h=h, rb=rb, att_u=att_u, att=att_d[h],
                                  hsl=hsl, b=b, i5=i5):
                        bc = bcp.tile([64, 512], BF16, name=f"bc_{b}_{i5}_{h}", tag="bc")
                        nc.gpsimd.dma_start(
                            out=bc[:, :], in_=rb[0:1, :].to_broadcast([64, 512])
                        )
                        nc.gpsimd.tensor_mul(att[:, hsl], att_u[0:64, :], bc[:, :])
                        if i5 % 2 == 1:
                            if b < B - 1:
                                nc.gpsimd.dma_start(
                                    out=send[b][i5 - 1:i5 + 1, h * 64:(h + 1) * 64, :]
                                        .rearrange("d p c -> p d c"),
                                    in_=att[:, :].rearrange("p (d c) -> p d c", d=2),
                                )
                            else:
                                for hf in range(2):
                                    nc.gpsimd.dma_start(
                                        out=send[b][hf][i5 - 1:i5 + 1,
                                                        h * 64:(h + 1) * 64, :]
                                            .rearrange("d p c -> p d c"),
                                        in_=att[:, :].rearrange(
                                            "p (d c) -> p d c", d=2
                                        )[:, :, hf * 256:(hf + 1) * 256],
                                    )
                    pend_norm.append(norm_tail)
                if i5 == NI - 1:
                    # batch boundary: flush immediately so the A2A can trigger
                    for fn in pend_norm:
                        fn()
                    pend_norm = []

            # ---- AllToAll for this batch: head-shard -> seq-shard. The
            # gather is emitted on the same gpsimd queue right after the
            # collective, so its trigger never waits (an A2A-gated gather
            # head-blocked an in-order queue for ~40us in v5).
            if b < B - 1:
                nc.gpsimd.collective_compute(
                    "AllToAll",
                    Alu.bypass,
                    replica_groups=[list(range(NCORES))],
                    ins=[send[b][:, :, :].opt()],
                    outs=[recv[b][:, :, :].opt()],
                )
                emit_rv_gather(b)
            else:
                for hf in range(2):
                    nc.gpsimd.collective_compute(
                        "AllToAll",
                        Alu.bypass,
                        replica_groups=[list(range(NCORES))],
                        ins=[send[b][hf][:, :, :].opt()],
                        outs=[recv[b][hf][:, :, :].opt()],
                    )
                    emit_rv_gather(b, hf)

        # ---- drain: outproj(2) chunks 3..7 cover A2A(3) trigger+wire, then
        # batch 3's out-projection (chunks 0..3 need only the first A2A half)
        for ob, ci in DRAIN_OP:
            emit_outproj_chunk(ob, ci)
        for chunk in range(8):
            emit_outproj_chunk(B - 1, chunk)


def prep_in_maps(x, y, Wq, bq, Wk, bk, Wv, bv, Wo, bo):
    f8 = ml_dtypes.float8_e4m3fn
    bf = ml_dtypes.bfloat16
    x = np.asarray(x, np.float32)
    y = np.asarray(y, np.float32)
    xt = np.ascontiguousarray(x.transpose(0, 2, 1)).reshape(B, KC, 128, SQ).astype(f8)
    yt = np.ascontiguousarray(y.transpose(0, 2, 1)).reshape(B, FC, 128, SK).astype(bf)
    wo = np.ascontiguousarray(np.asarray(Wo, np.float32).reshape(KC, 128, D)).astype(bf)
    bob = np.ascontiguousarray(
        np.broadcast_to(np.asarray(bo, np.float32)[None, :], (128, D))
    )
    in_maps = []
    for c in range(NCORES):
        cs = slice(c * 128, (c + 1) * 128)
        in_maps.append({
            "xt": xt,
            "yt": yt,
            # wq scaled x64 into fp8e4's normal range; bq pre-scaled to match
            # (the kernel multiplies the q psum by 0.125/64)
            "wq": np.ascontiguousarray(np.asarray(Wq, np.float32)[:, cs].reshape(KC, 128, 128) * 64.0).astype(f8),
            "wk": np.ascontiguousarray(np.asarray(Wk, np.float32)[:, cs].reshape(FC, 128, 128)).astype(bf),
            "wv": np.ascontiguousarray(np.asarray(Wv, np.float32)[:, cs].reshape(FC, 128, 128)).astype(bf),
            "wo": wo,
            "bq": np.ascontiguousarray(np.asarray(bq, np.float32)[cs].reshape(128, 1) * 64.0),
            "bk": np.ascontiguousarray(np.asarray(bk, np.float32)[cs].reshape(128, 1)),
            "bvb": np.ascontiguousarray(
                np.broadcast_to(np.asarray(bv, np.float32)[cs][None, :], (128, 128))
            ),
            "bob": bob,
        })
    return in_maps


_NC_CACHE = None


def get_nc():
    global _NC_CACHE
    if _NC_CACHE is None:
        _NC_CACHE = build_nc()
    return _NC_CACHE


def run(in_maps, **kwargs):
    nc = get_nc()
    return bass_utils.run_bass_kernel_spmd(
        nc, in_maps, core_ids=list(range(NCORES)), **kwargs
    )


def gather(results):
    full = np.empty((B, SQ, D), np.float32)
    for c in range(NCORES):
        full[:, c * SQL:(c + 1) * SQL, :] = results[c]["out"]
    return full


def kernel(**inputs):
    in_maps = prep_in_maps(**inputs)
    res = run(in_maps)
    return gather(res.results)


if __name__ == "__main__":
    nc = build_nc()
    print("build OK")


# revision 39
# speedup vs baseline: 1.1229x; 1.1229x over previous
"""Cross-attention (B=4, Sq=4096, Sk=1024, H=16, D=1024) on 8 TRN2 NeuronCores.

Sharding: tensor-parallel by heads. Core c owns heads (2c, 2c+1), i.e. columns
[128c, 128c+128) of Wq/Wk/Wv and rows [128c, 128c+128) of Wo.

v5 design notes (vs v2 baseline at ~476us):
  - q-proj runs fp8e4 DoubleRow (two 128-contraction chunks fused per MM,
    ~1.7x on that stream); wq is host-scaled x64 into fp8's normal range and
    the inverse scale folds into the existing qt evac multiply. Everything
    on the value path (k/v-proj, att@v, out-proj, A2A payload) stays bf16:
    each fp8e4 quantization of a random-sign operand costs ~2.5% rel error
    (signal and noise both scale as sqrt(N)), which a 2e-2 gate cannot
    absorb (full-fp8 v4 measured 4.4e-2). fp8 xt also halves x DMA traffic.
  - Startup reordered: minimal prefix (q(0,0), k(0,*), v(0,0..1)) before the
    first scores, everything else rides as fillers; first exp ~10us vs 38us.
  - DMA queues split: sync = bulk input loads + rv gathers + out stores;
    gpsimd = the self-contained normalization chain (rb/bc rides, norm
    multiplies, sends, collectives). In v2 a single in-order sync queue
    head-blocked latency-critical sends behind 2MB xt loads, starving ACT
    ~10us at every batch boundary; in v4 the rv gather's wait-on-A2A head-
    blocked the gpsimd queue for ~14us per boundary.
  - Out-proj chunks ride inside attention slots of later batches instead of
    a ~50us post-attention drain; only batch 3's out-proj (+A2A(3) wait)
    remains in the tail.

Host prep: x pre-transposed/chunked to fp8e4, y/W* to bf16; scores matmuls
bf16; all PSUM accumulation fp32; output fp32.
"""

import numpy as np
import ml_dtypes

import concourse.bass as bass
import concourse.mybir as mybir
from concourse import bacc, tile
from concourse import bass_utils

BF16 = mybir.dt.bfloat16
F32 = mybir.dt.float32
F8E4 = mybir.dt.float8e4

B = 4
SQ = 4096
SK = 1024
D = 1024
DC = 768
NCORES = 8
SQL = SQ // NCORES  # 512 output rows per batch per core
KC = D // 128       # 8 contraction chunks for q-proj / out-proj
FC = DC // 128      # 6 contraction chunks for k/v-proj
JC = SK // 128      # 8 key chunks
NI = SQ // 512      # 8 query blocks of 512 per batch

Exp = mybir.ActivationFunctionType.Exp
Alu = mybir.AluOpType
DR = mybir.MatmulPerfMode.DoubleRow


def build_nc():
    nc = bacc.Bacc(
        "TRN2",
        target_bir_lowering=False,
        debug=False,
        num_devices=NCORES,
    )

    xt = nc.dram_tensor("xt", [B, KC, 128, SQ], F8E4, kind="ExternalInput")
    yt = nc.dram_tensor("yt", [B, FC, 128, SK], BF16, kind="ExternalInput")
    wq = nc.dram_tensor("wq", [KC, 128, 128], F8E4, kind="ExternalInput")
    wk = nc.dram_tensor("wk", [FC, 128, 128], BF16, kind="ExternalInput")
    wv = nc.dram_tensor("wv", [FC, 128, 128], BF16, kind="ExternalInput")
    wo = nc.dram_tensor("wo", [KC, 128, D], BF16, kind="ExternalInput")
    bq = nc.dram_tensor("bq", [128, 1], F32, kind="ExternalInput")
    bk = nc.dram_tensor("bk", [128, 1], F32, kind="ExternalInput")
    bvb = nc.dram_tensor("bvb", [128, 128], F32, kind="ExternalInput")
    bob = nc.dram_tensor("bob", [128, D], F32, kind="ExternalInput")
    out = nc.dram_tensor("out", [B, SQL, D], F32, kind="ExternalOutput")

    # DRAM bounce buffers for the per-batch AllToAll. Per dest core:
    # rows 0:64 = head A vals, 64:128 = head B vals (already normalized).
    # The last batch's A2A is split into two query-half collectives so its
    # out-proj can overlap the second half's wire time; collectives need
    # contiguous buffers, so the halves are separate tensors.
    send = [
        nc.dram_tensor(f"a2a_send_{b}", [NCORES, 128, 512], BF16, kind="Internal")
        for b in range(B - 1)
    ] + [[
        nc.dram_tensor(f"a2a_send3_{hf}", [NCORES, 128, 256], BF16, kind="Internal")
        for hf in range(2)
    ]]
    recv = [
        nc.dram_tensor(f"a2a_recv_{b}", [NCORES, 128, 512], BF16, kind="Internal")
        for b in range(B - 1)
    ] + [[
        nc.dram_tensor(f"a2a_recv3_{hf}", [NCORES, 128, 256], BF16, kind="Internal")
        for hf in range(2)
    ]]

    with tile.TileContext(nc) as tc:
        _program(nc, tc, xt, yt, wq, wk, wv, wo, bq, bk, bvb, bob, out, send, recv)
    nc.finalize()
    return nc


def _program(nc, tc, xt, yt, wq, wk, wv, wo, bq, bk, bvb, bob, out, send, recv):
    from contextlib import ExitStack

    with ExitStack() as ctx:
        const = ctx.enter_context(tc.tile_pool(name="const", bufs=1))
        ytp = ctx.enter_context(tc.tile_pool(name="ytp", bufs=2))
        xtp = ctx.enter_context(tc.tile_pool(name="xtp", bufs=10))
        qtp = ctx.enter_context(tc.tile_pool(name="qtp", bufs=2))
        ktp = ctx.enter_context(tc.tile_pool(name="ktp", bufs=2))
        vtp = ctx.enter_context(tc.tile_pool(name="vtp", bufs=16))
        ep = ctx.enter_context(tc.tile_pool(name="ep", bufs=3))
        attp = ctx.enter_context(tc.tile_pool(name="attp", bufs=6))
        attup = ctx.enter_context(tc.tile_pool(name="attup", bufs=10))
        recp = ctx.enter_context(tc.tile_pool(name="recp", bufs=4))
        recbp = ctx.enter_context(tc.tile_pool(name="recbp", bufs=8))
        bcp = ctx.enter_context(tc.tile_pool(name="bcp", bufs=10))
        rvp = ctx.enter_context(tc.tile_pool(name="rvp", bufs=3))
        outp = ctx.enter_context(tc.tile_pool(name="outp", bufs=3))
        rbp = ctx.enter_context(tc.tile_pool(name="rbp", bufs=8, space="DRAM"))
        # PSUM: scores 2x2 banks + nout 2x1 + proj 2x1 = 8 banks
        scp = ctx.enter_context(tc.tile_pool(name="scp", bufs=2, space="PSUM"))
        noutp = ctx.enter_context(tc.tile_pool(name="noutp", bufs=2, space="PSUM"))
        projp = ctx.enter_context(tc.tile_pool(name="projp", bufs=2, space="PSUM"))

        # ---- ACT warmup: pull the exp table load off the critical path
        warm = const.tile([128, 1], F32, tag="warm")
        nc.vector.memset(warm[:, :], 0.0)
        warm_o = const.tile([128, 1], F32, tag="warmo")
        nc.scalar.activation(warm_o[:, :], warm[:, :], Exp)

        # ---- constants / weights resident in SBUF
        bq_sb = const.tile([128, 1], F32, tag="bq")
        nc.sync.dma_start(out=bq_sb[:, :], in_=bq[:, :])
        bk_sb = const.tile([128, 1], F32, tag="bk")
        nc.sync.dma_start(out=bk_sb[:, :], in_=bk[:, :])
        bvb_sb = const.tile([128, 128], F32, tag="bvb")
        nc.sync.dma_start(out=bvb_sb[:, :], in_=bvb[:, :])

        wq_sb = const.tile([128, KC, 128], F8E4, tag="wq")

        def emit_wq_load():
            nc.sync.dma_start(
                out=wq_sb[:, :, :],
                in_=wq[:, :, :].rearrange("k p c -> p k c"),
            )
        wk_sb = const.tile([128, FC, 128], BF16, tag="wk")
        wv_sb = const.tile([128, FC, 128], BF16, tag="wv")

        def emit_wkv_load():
            nc.sync.dma_start(
                out=wk_sb[:, :, :],
                in_=wk[:, :, :].rearrange("f p c -> p f c"),
            )
            nc.sync.dma_start(
                out=wv_sb[:, :, :],
                in_=wv[:, :, :].rearrange("f p c -> p f c"),
            )
        wo_sb = const.tile([128, KC, D], BF16, tag="wo")
        bob_sb = const.tile([128, D], F32, tag="bob")

        def emit_wo_load():
            nc.sync.dma_start(
                out=wo_sb[:, :, :],
                in_=wo[:, :, :].rearrange("k p c -> p k c"),
            )
            nc.sync.dma_start(out=bob_sb[:, :], in_=bob[:, :])

        yt_d = {}
        kt_d = {}
        qt_d = {}
        xt_d = {}
        v_tiles = {}
        att_d = {}
        rvs_d = {}
        o_d = {}

        def emit_yt_load(pb):
            t = ytp.tile([128, FC, SK], BF16, name=f"yt_{pb}", tag="yt")
            nc.sync.dma_start(
                out=t[:, :, :],
                in_=yt[pb, :, :, :].rearrange("f p c -> p f c"),
            )
            yt_d[pb] = t
            kt_d[pb] = ktp.tile([128, SK], BF16, name=f"kt_{pb}", tag="kt")
            qt_d[pb] = qtp.tile([128, SQ], BF16, name=f"qt_{pb}", tag="qt")

        def emit_xt_load(pb, i5):
            t = xtp.tile([128, KC, 512], F8E4, name=f"xt_{pb}_{i5}", tag="xt")
            nc.sync.dma_start(
                out=t[:, :, :],
                in_=xt[pb, :, :, i5 * 512:(i5 + 1) * 512].rearrange("k p c -> p k c"),
            )
            xt_d[(pb, i5)] = t

        def emit_xt_slot(slot):
            # absolute q-block slot -> (batch, i5)
            if slot < B * NI:
                emit_xt_load(slot // NI, slot % NI)

        def emit_k_chain(pb, j2):
            yt_sb = yt_d[pb]
            kps = projp.tile([128, 512], F32, name=f"kps_{pb}_{j2}", tag="proj")
            for fc in range(FC):
                nc.tensor.matmul(
                    kps[:, :],
                    lhsT=wk_sb[:, fc, :],
                    rhs=yt_sb[:, fc, j2 * 512:(j2 + 1) * 512],
                    start=(fc == 0),
                    stop=(fc == FC - 1),
                )
            nc.vector.tensor_scalar_add(
                kt_d[pb][:, j2 * 512:(j2 + 1) * 512], kps[:, :], bk_sb[:, :]
            )

        def emit_v_chain(pb, jc):
            # v_aug layout per tile [128, 130]:
            #   cols 0:64  = head-A values, col 64  = ones (A sums)
            #   cols 65:129 = head-B values, col 129 = ones (B sums)
            yt_sb = yt_d[pb]
            vps = projp.tile([128, 128], F32, name=f"vps_{pb}_{jc}", tag="proj")
            for fc in range(FC):
                nc.tensor.matmul(
                    vps[:, :],
                    lhsT=yt_sb[:, fc, jc * 128:(jc + 1) * 128],
                    rhs=wv_sb[:, fc, :],
                    start=(fc == 0),
                    stop=(fc == FC - 1),
                )
            v_t = vtp.tile([128, 130], BF16, name=f"v_{pb}_{jc}", tag="vt")
            nc.vector.tensor_tensor(
                out=v_t[:, 0:130].rearrange("p (h x) -> p h x", h=2)[:, :, 0:64],
                in0=vps[:, :].rearrange("p (h x) -> p h x", h=2),
                in1=bvb_sb[:, :].rearrange("p (h x) -> p h x", h=2),
                op=Alu.add,
            )
            nc.vector.memset(v_t[:, 64:65], 1.0)
            nc.vector.memset(v_t[:, 129:130], 1.0)
            v_tiles[(pb, jc)] = v_t

        def emit_q_chain(pb, i5):
            xt_sb = xt_d.pop((pb, i5))
            qps = projp.tile([128, 512], F32, name=f"qps_{pb}_{i5}", tag="proj")
            for kp in range(KC // 2):
                nc.tensor.matmul(
                    qps[:, :],
                    lhsT=wq_sb[:, 2 * kp:2 * kp + 2, :],
                    rhs=xt_sb[:, 2 * kp:2 * kp + 2, :],
                    start=(kp == 0),
                    stop=(kp == KC // 2 - 1),
                    perf_mode=DR,
                )
            # wq is host-scaled x64 into fp8's normal range; bq is pre-scaled
            # x64 on the host so (psum + bq*64) * (0.125/64) = (q + bq) / 8
            nc.vector.tensor_scalar(
                out=qt_d[pb][:, i5 * 512:(i5 + 1) * 512],
                in0=qps[:, :],
                scalar1=bq_sb[:, :],
                scalar2=0.125 / 64.0,
                op0=Alu.add,
                op1=Alu.mult,
            )

        def emit_rv_gather(ob, hf=None):
            # Emitted on the gpsimd queue right AFTER the collective, so the
            # trigger never waits: an A2A-gated gather on an in-order queue
            # head-blocks everything behind it (40us stall in v5).
            if ob not in rvs_d:
                rvs_d[ob] = rvp.tile([128, KC, 512], BF16, name=f"rv_{ob}", tag="rv")
            if hf is None:
                nc.gpsimd.dma_start(
                    out=rvs_d[ob][:, :, :],
                    in_=recv[ob][:, :, :].rearrange("k p c -> p k c"),
                )
            else:
                nc.gpsimd.dma_start(
                    out=rvs_d[ob][:, :, hf * 256:(hf + 1) * 256],
                    in_=recv[ob][hf][:, :, :].rearrange("k p c -> p k c"),
                )

        def emit_outproj_chunk(ob, chunk):
            i1, eh = divmod(chunk, 2)
            rvs = rvs_d[ob]
            ops = projp.tile([128, 512], F32, name=f"ops_{ob}_{chunk}", tag="proj")
            for cc in range(KC):
                nc.tensor.matmul(
                    ops[:, :],
                    lhsT=rvs[:, cc, i1 * 128:(i1 + 1) * 128],
                    rhs=wo_sb[:, cc, eh * 512:(eh + 1) * 512],
                    start=(cc == 0),
                    stop=(cc == KC - 1),
                )
            if eh == 0:
                o_d[(ob, i1)] = outp.tile(
                    [128, 1024], F32, name=f"o_{ob}_{i1}", tag="o"
                )
            o_t = o_d[(ob, i1)]
            nc.vector.tensor_add(
                o_t[:, eh * 512:(eh + 1) * 512], ops[:, :],
                bob_sb[:, eh * 512:(eh + 1) * 512],
            )
            if eh == 1:
                nc.sync.dma_start(
                    out=out[ob, i1 * 128:(i1 + 1) * 128, :], in_=o_t[:, :]
                )

        # ---- startup: minimal prefix so the first exp lands ~10us in. The
        # rest of batch 0's prep rides as fillers inside its attention loop.
        emit_wq_load()
        emit_xt_slot(0)
        emit_yt_load(0)
        emit_wkv_load()
        emit_xt_slot(1)
        emit_xt_slot(2)
        emit_q_chain(0, 0)
        emit_k_chain(0, 0)
        emit_k_chain(0, 1)
        for jc in range(6):
            emit_v_chain(0, jc)

        # ---- filler schedule, keyed by absolute slot (b*8+i5).
        # q(0,t) at slot t-1; q(1,t) shifted +2 (slots 2..9); q(b>=2,t) at
        # slot (b-1)*8+t. v(0,2..7) inside slot 0; v(1,t) shifted like q.
        from collections import defaultdict
        fills = defaultdict(list)

        for t in range(1, NI):
            fills[t - 1].append(lambda j=t: emit_q_chain(0, j))
        for jc in range(6, JC):
            fills[0].append(lambda j=jc: emit_v_chain(0, j))
        fills[1].append(emit_wo_load)
        for t in range(NI):
            fills[t + 2].append(lambda j=t: emit_q_chain(1, j))
            fills[t].append(lambda j=t: emit_v_chain(1, j))
        fills[2].append(lambda: emit_k_chain(1, 0))
        fills[3].append(lambda: emit_k_chain(1, 1))
        for b in range(2, B):
            for t in range(NI):
                fills[(b - 1) * 8 + t].append(lambda pb=b, j=t: emit_q_chain(pb, j))
                fills[(b - 1) * 8 + t].append(lambda pb=b, j=t: emit_v_chain(pb, j))
            fills[(b - 1) * 8 + 0].append(lambda pb=b: emit_k_chain(pb, 0))
            fills[(b - 1) * 8 + 1].append(lambda pb=b: emit_k_chain(pb, 1))
        # yt(b) loads: yt(0), yt(1) at startup; yt(2) at slot 6; yt(3) at 14
        emit_yt_load(1)
        fills[6].append(lambda: emit_yt_load(2))
        fills[14].append(lambda: emit_yt_load(3))
        # xt prefetch, 2-slot lead over consumption: blocks 0-2 at startup;
        # b0 blocks 3-7 consumed at slot s-1; b1 blocks 8-15 consumed at
        # slot s-6 (q(1,t) shifted +2); b>=2 blocks consumed at slot s-8
        for s in range(3, 8):
            fills[s - 3].append(lambda ss=s: emit_xt_slot(ss))
        for s in range(8, 16):
            fills[s - 8].append(lambda ss=s: emit_xt_slot(ss))
        for T in range(6, B * NI):
            fills[T].append(lambda ss=T + 10: emit_xt_slot(ss))

        # out-proj chunk placement: the tile scheduler hoists out-proj
        # LDWEIGHTS (whose only dependency is the gathered rv tile) several
        # slots ahead in the in-order tensor queue. A chunk emitted earlier
        # than ~2 batches after its A2A gets hoisted to before the A2A even
        # fires and head-blocks the queue ~40us. So outproj(0) rides in
        # batch 3 and outproj(1,2) in the drain, where their rv tiles are
        # long since ready no matter how far the LDW is hoisted.
        OP_SCHED = {(3, i): [(0, i)] for i in range(8)}
        DRAIN_OP = [(ob, ci) for ob in (1, 2) for ci in range(8)]

        pend_norm = []

        for b in range(B):
            kt_sb = kt_d[b]
            qt_sb = qt_d[b]

            for i5 in range(NI):
                slot = b * NI + i5
                fill = list(fills.pop(slot, ()))
                for ob, cc in OP_SCHED.get((b, i5), []):
                    fill.append(lambda ob=ob, cc=cc: emit_outproj_chunk(ob, cc))

                isl = slice(i5 * 512, (i5 + 1) * 512)
                na = noutp.tile([65, 512], F32, name=f"na_{b}_{i5}", tag="nout")
                nb = noutp.tile([65, 512], F32, name=f"nb_{b}_{i5}", tag="nout")

                def emit_scores(jc):
                    sc = scp.tile([128, 1024], F32, name=f"sc_{b}_{i5}_{jc}", tag="sc")
                    jsl = slice(jc * 128, (jc + 1) * 128)
                    # scoresT for both heads, row-tiled (K=64 each, concurrent)
                    nc.tensor.matmul(
                        sc[:, 0:512],
                        lhsT=kt_sb[0:64, jsl],
                        rhs=qt_sb[0:64, isl],
                        start=True, stop=True,
                    )
                    nc.tensor.matmul(
                        sc[:, 512:1024],
                        lhsT=kt_sb[64:128, jsl],
                        rhs=qt_sb[64:128, isl],
                        start=True, stop=True,
                    )
                    e_t = ep.tile([128, 1024], BF16, name=f"e_{b}_{i5}_{jc}", tag="e")
                    nc.scalar.activation(e_t[:, :], sc[:, :], Exp)
                    return e_t

                # software-pipelined over jc: scores(jc+1) FIRST in each slot
                # (so exp never queues behind filler matmuls), then fillers
                # run while ACT computes exp(jc), then av(jc) when it lands
                e_cur = emit_scores(0)
                nfill = len(fill)
                for jc in range(JC):
                    e_next = emit_scores(jc + 1) if jc + 1 < JC else None
                    f0 = jc * nfill // JC
                    f1 = (jc + 1) * nfill // JC
                    for f in fill[f0:f1]:
                        f()
                    v_t = v_tiles[(b, jc)]
                    nc.tensor.matmul(
                        na[:, :],
                        lhsT=v_t[:, 0:65],
                        rhs=e_cur[:, 0:512],
                        start=(jc == 0),
                        stop=(jc == JC - 1),
                    )
                    nc.tensor.matmul(
                        nb[:, :],
                        lhsT=v_t[:, 65:130],
                        rhs=e_cur[:, 512:1024],
                        start=(jc == 0),
                        stop=(jc == JC - 1),
                    )
                    e_cur = e_next

                # evacuate nout psum, normalize by the sums row, stage fp8 att
                # tiles; one send DMA per (i5-pair, head)
                if i5 % 2 == 0:
                    att_d[0] = attp.tile([64, 1024], BF16, name=f"attA_{b}_{i5}", tag="att")
                    att_d[1] = attp.tile([64, 1024], BF16, name=f"attB_{b}_{i5}", tag="att")
                hsl = slice((i5 % 2) * 512, (i5 % 2) * 512 + 512)
                # emit the PREVIOUS i5's deferred broadcast+multiply first:
                # by now its rb ride has landed, so the gpsimd queue never
                # stalls on the DRAM round-trip
                for fn in pend_norm:
                    fn()
                pend_norm = []
                for h, nres in ((0, na), (1, nb)):
                    att_u = attup.tile([65, 512], BF16, name=f"au_{b}_{i5}_{h}", tag="au")
                    nc.vector.tensor_copy(att_u[:, :], nres[:, :])
                    rec = recp.tile([65, 512], F32, name=f"rec_{b}_{i5}_{h}", tag="rec")
                    nc.vector.reciprocal_approx_fast(out=rec[:, :], in_=nres[:, :])
                    recb = recbp.tile([65, 512], BF16, name=f"rcb_{b}_{i5}_{h}", tag="rcb")
                    nc.vector.tensor_copy(recb[:, :], rec[:, :])
                    rb = rbp.tile([1, 512], BF16, name=f"rb_{b}_{i5}_{h}", tag="rb")
                    nc.gpsimd.dma_start(out=rb[:, :], in_=recb[64:65, :])

                    def norm_tail(h=h, rb=rb, att_u=att_u, att=att_d[h],
                                  hsl=hsl, b=b, i5=i5):
                        bc = bcp.tile([64, 512], BF16, name=f"bc_{b}_{i5}_{h}", tag="bc")
                        nc.gpsimd.dma_start(
                            out=bc[:, :], in_=rb[0:1, :].to_broadcast([64, 512])
                        )
                        nc.gpsimd.tensor_mul(att[:, hsl], att_u[0:64, :], bc[:, :])
                        if i5 % 2 == 1:
                            if b < B - 1:
                                nc.gpsimd.dma_start(
                                    out=send[b][i5 - 1:i5 + 1, h * 64:(h + 1) * 64, :]
                                        .rearrange("d p c -> p d c"),
                                    in_=att[:, :].rearrange("p (d c) -> p d c", d=2),
                                )
                            else:
                                for hf in range(2):
                                    nc.gpsimd.dma_start(
                                        out=send[b][hf][i5 - 1:i5 + 1,
                                                        h * 64:(h + 1) * 64, :]
                                            .rearrange("d p c -> p d c"),
                                        in_=att[:, :].rearrange(
                                            "p (d c) -> p d c", d=2
                                        )[:, :, hf * 256:(hf + 1) * 256],
                                    )
                    pend_norm.append(norm_tail)
                if i5 == NI - 1:
                    # batch boundary: flush immediately so the A2A can trigger
                    for fn in pend_norm:
                        fn()
                    pend_norm = []

            # ---- AllToAll for this batch: head-shard -> seq-shard. The
            # gather is emitted on the same gpsimd queue right after the
            # collective, so its trigger never waits (an A2A-gated gather
            # head-blocked an in-order queue for ~40us in v5).
            if b < B - 1:
                nc.gpsimd.collective_compute(
                    "AllToAll",
                    Alu.bypass,
                    replica_groups=[list(range(NCORES))],
                    ins=[send[b][:, :, :].opt()],
                    outs=[recv[b][:, :, :].opt()],
                )
                emit_rv_gather(b)
            else:
                for hf in range(2):
                    nc.gpsimd.collective_compute(
                        "AllToAll",
                        Alu.bypass,
                        replica_groups=[list(range(NCORES))],
                        ins=[send[b][hf][:, :, :].opt()],
                        outs=[recv[b][hf][:, :, :].opt()],
                    )
                    emit_rv_gather(b, hf)

        # ---- drain: outproj(2) chunks 3..7 cover A2A(3) trigger+wire, then
        # batch 3's out-projection (chunks 0..3 need only the first A2A half)
        for ob, ci in DRAIN_OP:
            emit_outproj_chunk(ob, ci)
        for chunk in range(8):
            emit_outproj_chunk(B - 1, chunk)


def prep_in_maps(x, y, Wq, bq, Wk, bk, Wv, bv, Wo, bo):
    f8 = ml_dtypes.float8_e4m3fn
    bf = ml_dtypes.bfloat16
    x = np.asarray(x, np.float32)
    y = np.asarray(y, np.float32)
    xt = np.ascontiguousarray(x.transpose(0, 2, 1)).reshape(B, KC, 128, SQ).astype(f8)
    yt = np.ascontiguousarray(y.transpose(0, 2, 1)).reshape(B, FC, 128, SK).astype(bf)
    wo = np.ascontiguousarray(np.asarray(Wo, np.float32).reshape(KC, 128, D)).astype(bf)
    bob = np.ascontiguousarray(
        np.broadcast_to(np.asarray(bo, np.float32)[None, :], (128, D))
    )
    in_maps = []
    for c in range(NCORES):
        cs = slice(c * 128, (c + 1) * 128)
        in_maps.append({
            "xt": xt,
            "yt": yt,
            # wq scaled x64 into fp8e4's normal range; bq pre-scaled to match
            # (the kernel multiplies the q psum by 0.125/64)
            "wq": np.ascontiguousarray(np.asarray(Wq, np.float32)[:, cs].reshape(KC, 128, 128) * 64.0).astype(f8),
            "wk": np.ascontiguousarray(np.asarray(Wk, np.float32)[:, cs].reshape(FC, 128, 128)).astype(bf),
            "wv": np.ascontiguousarray(np.asarray(Wv, np.float32)[:, cs].reshape(FC, 128, 128)).astype(bf),
            "wo": wo,
            "bq": np.ascontiguousarray(np.asarray(bq, np.float32)[cs].reshape(128, 1) * 64.0),
            "bk": np.ascontiguousarray(np.asarray(bk, np.float32)[cs].reshape(128, 1)),
            "bvb": np.ascontiguousarray(
                np.broadcast_to(np.asarray(bv, np.float32)[cs][None, :], (128, 128))
            ),
            "bob": bob,
        })
    return in_maps


_NC_CACHE = None


def get_nc():
    global _NC_CACHE
    if _NC_CACHE is None:
        _NC_CACHE = build_nc()
    return _NC_CACHE


def run(in_maps, **kwargs):
    nc = get_nc()
    return bass_utils.run_bass_kernel_spmd(
        nc, in_maps, core_ids=list(range(NCORES)), **kwargs
    )


def gather(results):
    full = np.empty((B, SQ, D), np.float32)
    for c in range(NCORES):
        full[:, c * SQL:(c + 1) * SQL, :] = results[c]["out"]
    return full


def kernel(**inputs):
    in_maps = prep_in_maps(**inputs)
    res = run(in_maps)
    return gather(res.results)


if __name__ == "__main__":
    nc = build_nc()
    print("build OK")


# revision 43
# speedup vs baseline: 1.1685x; 1.0406x over previous
"""Cross-attention (B=4, Sq=4096, Sk=1024, H=16, D=1024) on 8 TRN2 NeuronCores.

Sharding: tensor-parallel by heads. Core c owns heads (2c, 2c+1), i.e. columns
[128c, 128c+128) of Wq/Wk/Wv and rows [128c, 128c+128) of Wo.

v5 design notes (vs v2 baseline at ~476us):
  - q-proj runs fp8e4 DoubleRow (two 128-contraction chunks fused per MM,
    ~1.7x on that stream); wq is host-scaled x64 into fp8's normal range and
    the inverse scale folds into the existing qt evac multiply. Everything
    on the value path (k/v-proj, att@v, out-proj, A2A payload) stays bf16:
    each fp8e4 quantization of a random-sign operand costs ~2.5% rel error
    (signal and noise both scale as sqrt(N)), which a 2e-2 gate cannot
    absorb (full-fp8 v4 measured 4.4e-2). fp8 xt also halves x DMA traffic.
  - Startup reordered: minimal prefix (q(0,0), k(0,*), v(0,0..1)) before the
    first scores, everything else rides as fillers; first exp ~10us vs 38us.
  - DMA queues split: sync = bulk input loads + rv gathers + out stores;
    gpsimd = the self-contained normalization chain (rb/bc rides, norm
    multiplies, sends, collectives). In v2 a single in-order sync queue
    head-blocked latency-critical sends behind 2MB xt loads, starving ACT
    ~10us at every batch boundary; in v4 the rv gather's wait-on-A2A head-
    blocked the gpsimd queue for ~14us per boundary.
  - Out-proj chunks ride inside attention slots of later batches instead of
    a ~50us post-attention drain; only batch 3's out-proj (+A2A(3) wait)
    remains in the tail.

Host prep: x pre-transposed/chunked to fp8e4, y/W* to bf16; scores matmuls
bf16; all PSUM accumulation fp32; output fp32.
"""

import numpy as np
import ml_dtypes

import concourse.bass as bass
import concourse.mybir as mybir
from concourse import bacc, tile
from concourse import bass_utils

BF16 = mybir.dt.bfloat16
F32 = mybir.dt.float32
F8E4 = mybir.dt.float8e4

B = 4
SQ = 4096
SK = 1024
D = 1024
DC = 768
NCORES = 8
SQL = SQ // NCORES  # 512 output rows per batch per core
KC = D // 128       # 8 contraction chunks for q-proj / out-proj
FC = DC // 128      # 6 contraction chunks for k/v-proj
JC = SK // 128      # 8 key chunks
NI = SQ // 512      # 8 query blocks of 512 per batch

Exp = mybir.ActivationFunctionType.Exp
Alu = mybir.AluOpType
DR = mybir.MatmulPerfMode.DoubleRow


def build_nc():
    nc = bacc.Bacc(
        "TRN2",
        target_bir_lowering=False,
        debug=False,
        num_devices=NCORES,
    )

    xt = nc.dram_tensor("xt", [B, KC, 128, SQ], F8E4, kind="ExternalInput")
    yt = nc.dram_tensor("yt", [B, FC, 128, SK], BF16, kind="ExternalInput")
    wq = nc.dram_tensor("wq", [KC, 128, 128], F8E4, kind="ExternalInput")
    wk = nc.dram_tensor("wk", [FC, 128, 128], BF16, kind="ExternalInput")
    wv = nc.dram_tensor("wv", [FC, 128, 128], BF16, kind="ExternalInput")
    wo = nc.dram_tensor("wo", [KC, 128, D], BF16, kind="ExternalInput")
    bq = nc.dram_tensor("bq", [128, 1], F32, kind="ExternalInput")
    bk = nc.dram_tensor("bk", [128, 1], F32, kind="ExternalInput")
    bvb = nc.dram_tensor("bvb", [128, 128], F32, kind="ExternalInput")
    bob = nc.dram_tensor("bob", [128, D], F32, kind="ExternalInput")
    out = nc.dram_tensor("out", [B, SQL, D], F32, kind="ExternalOutput")

    # DRAM bounce buffers for the per-batch AllToAll. Per dest core:
    # rows 0:64 = head A vals, 64:128 = head B vals (already normalized).
    # The last batch's A2A is split into two query-half collectives so its
    # out-proj can overlap the second half's wire time; collectives need
    # contiguous buffers, so the halves are separate tensors.
    send = [
        nc.dram_tensor(f"a2a_send_{b}", [NCORES, 128, 512], BF16, kind="Internal")
        for b in range(B - 1)
    ] + [[
        nc.dram_tensor(f"a2a_send3_{hf}", [NCORES, 128, 256], BF16, kind="Internal")
        for hf in range(2)
    ]]
    recv = [
        nc.dram_tensor(f"a2a_recv_{b}", [NCORES, 128, 512], BF16, kind="Internal")
        for b in range(B - 1)
    ] + [[
        nc.dram_tensor(f"a2a_recv3_{hf}", [NCORES, 128, 256], BF16, kind="Internal")
        for hf in range(2)
    ]]

    with tile.TileContext(nc) as tc:
        _program(nc, tc, xt, yt, wq, wk, wv, wo, bq, bk, bvb, bob, out, send, recv)
    nc.finalize()
    return nc


def _program(nc, tc, xt, yt, wq, wk, wv, wo, bq, bk, bvb, bob, out, send, recv):
    from contextlib import ExitStack

    with ExitStack() as ctx:
        const = ctx.enter_context(tc.tile_pool(name="const", bufs=1))
        ytp = ctx.enter_context(tc.tile_pool(name="ytp", bufs=2))
        xtp = ctx.enter_context(tc.tile_pool(name="xtp", bufs=10))
        qtp = ctx.enter_context(tc.tile_pool(name="qtp", bufs=2))
        ktp = ctx.enter_context(tc.tile_pool(name="ktp", bufs=2))
        vtp = ctx.enter_context(tc.tile_pool(name="vtp", bufs=16))
        ep = ctx.enter_context(tc.tile_pool(name="ep", bufs=3))
        attp = ctx.enter_context(tc.tile_pool(name="attp", bufs=6))
        attup = ctx.enter_context(tc.tile_pool(name="attup", bufs=10))
        recp = ctx.enter_context(tc.tile_pool(name="recp", bufs=4))
        recbp = ctx.enter_context(tc.tile_pool(name="recbp", bufs=8))
        bcp = ctx.enter_context(tc.tile_pool(name="bcp", bufs=10))
        rvp = ctx.enter_context(tc.tile_pool(name="rvp", bufs=3))
        outp = ctx.enter_context(tc.tile_pool(name="outp", bufs=3))
        rbp = ctx.enter_context(tc.tile_pool(name="rbp", bufs=8, space="DRAM"))
        # PSUM: scores 2x2 banks + nout 2x1 + proj 2x1 = 8 banks
        scp = ctx.enter_context(tc.tile_pool(name="scp", bufs=2, space="PSUM"))
        noutp = ctx.enter_context(tc.tile_pool(name="noutp", bufs=2, space="PSUM"))
        projp = ctx.enter_context(tc.tile_pool(name="projp", bufs=2, space="PSUM"))

        # ---- ACT warmup: pull the exp table load off the critical path
        warm = const.tile([128, 1], F32, tag="warm")
        nc.vector.memset(warm[:, :], 0.0)
        warm_o = const.tile([128, 1], F32, tag="warmo")
        nc.scalar.activation(warm_o[:, :], warm[:, :], Exp)

        # ---- PE warmup: ~4us of dependency-free dummy matmuls overlap the
        # startup DMA wait (PE-idle otherwise), so the HAM clock gate is at
        # full rate before the first real projection chain instead of the
        # first ~20us running at the cold 1.2GHz state.
        wpe = const.tile([128, 512], BF16, tag="wpe")
        nc.vector.memset(wpe[:, :], 0.0)
        wps = projp.tile([128, 512], F32, name="wps", tag="proj")
        for wi in range(10):
            nc.tensor.matmul(
                wps[:, :], lhsT=wpe[:, 0:128], rhs=wpe[:, :],
                start=(wi == 0), stop=(wi == 9),
            )

        # ---- constants / weights resident in SBUF
        bq_sb = const.tile([128, 1], F32, tag="bq")
        nc.sync.dma_start(out=bq_sb[:, :], in_=bq[:, :])
        bk_sb = const.tile([128, 1], F32, tag="bk")
        nc.sync.dma_start(out=bk_sb[:, :], in_=bk[:, :])
        bvb_sb = const.tile([128, 128], F32, tag="bvb")
        nc.sync.dma_start(out=bvb_sb[:, :], in_=bvb[:, :])

        wq_sb = const.tile([128, KC, 128], F8E4, tag="wq")

        def emit_wq_load():
            nc.sync.dma_start(
                out=wq_sb[:, :, :],
                in_=wq[:, :, :].rearrange("k p c -> p k c"),
            )
        wk_sb = const.tile([128, FC, 128], BF16, tag="wk")
        wv_sb = const.tile([128, FC, 128], BF16, tag="wv")

        def emit_wkv_load():
            nc.sync.dma_start(
                out=wk_sb[:, :, :],
                in_=wk[:, :, :].rearrange("f p c -> p f c"),
            )
            nc.sync.dma_start(
                out=wv_sb[:, :, :],
                in_=wv[:, :, :].rearrange("f p c -> p f c"),
            )
        wo_sb = const.tile([128, KC, D], BF16, tag="wo")
        bob_sb = const.tile([128, D], F32, tag="bob")

        def emit_wo_load():
            nc.sync.dma_start(
                out=wo_sb[:, :, :],
                in_=wo[:, :, :].rearrange("k p c -> p k c"),
            )
            nc.sync.dma_start(out=bob_sb[:, :], in_=bob[:, :])

        yt_d = {}
        kt_d = {}
        qt_d = {}
        xt_d = {}
        v_tiles = {}
        att_d = {}
        rvs_d = {}
        o_d = {}

        def emit_yt_load(pb):
            t = ytp.tile([128, FC, SK], BF16, name=f"yt_{pb}", tag="yt")
            nc.sync.dma_start(
                out=t[:, :, :],
                in_=yt[pb, :, :, :].rearrange("f p c -> p f c"),
            )
            yt_d[pb] = t
            kt_d[pb] = ktp.tile([128, SK], BF16, name=f"kt_{pb}", tag="kt")
            qt_d[pb] = qtp.tile([128, SQ], BF16, name=f"qt_{pb}", tag="qt")

        def emit_xt_load(pb, i5):
            t = xtp.tile([128, KC, 512], F8E4, name=f"xt_{pb}_{i5}", tag="xt")
            nc.sync.dma_start(
                out=t[:, :, :],
                in_=xt[pb, :, :, i5 * 512:(i5 + 1) * 512].rearrange("k p c -> p k c"),
            )
            xt_d[(pb, i5)] = t

        def emit_xt_slot(slot):
            # absolute q-block slot -> (batch, i5)
            if slot < B * NI:
                emit_xt_load(slot // NI, slot % NI)

        def emit_k_chain(pb, j2):
            yt_sb = yt_d[pb]
            kps = projp.tile([128, 512], F32, name=f"kps_{pb}_{j2}", tag="proj")
            for fc in range(FC):
                nc.tensor.matmul(
                    kps[:, :],
                    lhsT=wk_sb[:, fc, :],
                    rhs=yt_sb[:, fc, j2 * 512:(j2 + 1) * 512],
                    start=(fc == 0),
                    stop=(fc == FC - 1),
                )
            nc.vector.tensor_scalar_add(
                kt_d[pb][:, j2 * 512:(j2 + 1) * 512], kps[:, :], bk_sb[:, :]
            )

        def emit_v_chain(pb, jc):
            # v_aug layout per tile [128, 130]:
            #   cols 0:64  = head-A values, col 64  = ones (A sums)
            #   cols 65:129 = head-B values, col 129 = ones (B sums)
            yt_sb = yt_d[pb]
            vps = projp.tile([128, 128], F32, name=f"vps_{pb}_{jc}", tag="proj")
            for fc in range(FC):
                nc.tensor.matmul(
                    vps[:, :],
                    lhsT=yt_sb[:, fc, jc * 128:(jc + 1) * 128],
                    rhs=wv_sb[:, fc, :],
                    start=(fc == 0),
                    stop=(fc == FC - 1),
                )
            v_t = vtp.tile([128, 130], BF16, name=f"v_{pb}_{jc}", tag="vt")
            nc.vector.tensor_tensor(
                out=v_t[:, 0:130].rearrange("p (h x) -> p h x", h=2)[:, :, 0:64],
                in0=vps[:, :].rearrange("p (h x) -> p h x", h=2),
                in1=bvb_sb[:, :].rearrange("p (h x) -> p h x", h=2),
                op=Alu.add,
            )
            nc.vector.memset(v_t[:, 64:65], 1.0)
            nc.vector.memset(v_t[:, 129:130], 1.0)
            v_tiles[(pb, jc)] = v_t

        def emit_q_chain(pb, i5):
            xt_sb = xt_d.pop((pb, i5))
            qps = projp.tile([128, 512], F32, name=f"qps_{pb}_{i5}", tag="proj")
            for kp in range(KC // 2):
                nc.tensor.matmul(
                    qps[:, :],
                    lhsT=wq_sb[:, 2 * kp:2 * kp + 2, :],
                    rhs=xt_sb[:, 2 * kp:2 * kp + 2, :],
                    start=(kp == 0),
                    stop=(kp == KC // 2 - 1),
                    perf_mode=DR,
                )
            # wq is host-scaled x64 into fp8's normal range; bq is pre-scaled
            # x64 on the host so (psum + bq*64) * (0.125/64) = (q + bq) / 8
            nc.vector.tensor_scalar(
                out=qt_d[pb][:, i5 * 512:(i5 + 1) * 512],
                in0=qps[:, :],
                scalar1=bq_sb[:, :],
                scalar2=0.125 / 64.0,
                op0=Alu.add,
                op1=Alu.mult,
            )

        def emit_rv_gather(ob, hf=None):
            # Emitted on the gpsimd queue right AFTER the collective, so the
            # trigger never waits: an A2A-gated gather on an in-order queue
            # head-blocks everything behind it (40us stall in v5).
            if ob not in rvs_d:
                rvs_d[ob] = rvp.tile([128, KC, 512], BF16, name=f"rv_{ob}", tag="rv")
            if hf is None:
                nc.gpsimd.dma_start(
                    out=rvs_d[ob][:, :, :],
                    in_=recv[ob][:, :, :].rearrange("k p c -> p k c"),
                )
            else:
                nc.gpsimd.dma_start(
                    out=rvs_d[ob][:, :, hf * 256:(hf + 1) * 256],
                    in_=recv[ob][hf][:, :, :].rearrange("k p c -> p k c"),
                )

        def emit_outproj_chunk(ob, chunk):
            i1, eh = divmod(chunk, 2)
            rvs = rvs_d[ob]
            ops = projp.tile([128, 512], F32, name=f"ops_{ob}_{chunk}", tag="proj")
            for cc in range(KC):
                nc.tensor.matmul(
                    ops[:, :],
                    lhsT=rvs[:, cc, i1 * 128:(i1 + 1) * 128],
                    rhs=wo_sb[:, cc, eh * 512:(eh + 1) * 512],
                    start=(cc == 0),
                    stop=(cc == KC - 1),
                )
            if eh == 0:
                o_d[(ob, i1)] = outp.tile(
                    [128, 1024], F32, name=f"o_{ob}_{i1}", tag="o"
                )
            o_t = o_d[(ob, i1)]
            nc.vector.tensor_add(
                o_t[:, eh * 512:(eh + 1) * 512], ops[:, :],
                bob_sb[:, eh * 512:(eh + 1) * 512],
            )
            if eh == 1:
                nc.sync.dma_start(
                    out=out[ob, i1 * 128:(i1 + 1) * 128, :], in_=o_t[:, :]
                )

        # ---- startup: minimal prefix so the first exp lands ~10us in. The
        # rest of batch 0's prep rides as fillers inside its attention loop.
        emit_wq_load()
        emit_xt_slot(0)
        emit_yt_load(0)
        emit_wkv_load()
        emit_xt_slot(1)
        emit_xt_slot(2)
        emit_q_chain(0, 0)
        emit_k_chain(0, 0)
        emit_k_chain(0, 1)
        for jc in range(6):
            emit_v_chain(0, jc)

        # ---- filler schedule, keyed by absolute slot (b*8+i5).
        # q(0,t) at slot t-1; q(1,t) shifted +2 (slots 2..9); q(b>=2,t) at
        # slot (b-1)*8+t. v(0,2..7) inside slot 0; v(1,t) shifted like q.
        from collections import defaultdict
        fills = defaultdict(list)

        for t in range(1, NI):
            fills[t - 1].append(lambda j=t: emit_q_chain(0, j))
        for jc in range(6, JC):
            fills[0].append(lambda j=jc: emit_v_chain(0, j))
        fills[1].append(emit_wo_load)
        for t in range(NI):
            fills[t + 2].append(lambda j=t: emit_q_chain(1, j))
            fills[t].append(lambda j=t: emit_v_chain(1, j))
        fills[2].append(lambda: emit_k_chain(1, 0))
        fills[3].append(lambda: emit_k_chain(1, 1))
        for b in range(2, B):
            for t in range(NI):
                fills[(b - 1) * 8 + t].append(lambda pb=b, j=t: emit_q_chain(pb, j))
                fills[(b - 1) * 8 + t].append(lambda pb=b, j=t: emit_v_chain(pb, j))
            fills[(b - 1) * 8 + 0].append(lambda pb=b: emit_k_chain(pb, 0))
            fills[(b - 1) * 8 + 1].append(lambda pb=b: emit_k_chain(pb, 1))
        # yt(b) loads: yt(0), yt(1) at startup; yt(2) at slot 6; yt(3) at 14
        emit_yt_load(1)
        fills[6].append(lambda: emit_yt_load(2))
        fills[14].append(lambda: emit_yt_load(3))
        # xt prefetch, 2-slot lead over consumption: blocks 0-2 at startup;
        # b0 blocks 3-7 consumed at slot s-1; b1 blocks 8-15 consumed at
        # slot s-6 (q(1,t) shifted +2); b>=2 blocks consumed at slot s-8
        for s in range(3, 8):
            fills[s - 3].append(lambda ss=s: emit_xt_slot(ss))
        for s in range(8, 16):
            fills[s - 8].append(lambda ss=s: emit_xt_slot(ss))
        for T in range(6, B * NI):
            fills[T].append(lambda ss=T + 10: emit_xt_slot(ss))

        # out-proj chunk placement: the tile scheduler hoists out-proj
        # LDWEIGHTS (whose only dependency is the gathered rv tile) several
        # slots ahead in the in-order tensor queue. A chunk emitted earlier
        # than ~2 batches after its A2A gets hoisted to before the A2A even
        # fires and head-blocks the queue ~40us. So outproj(0) rides in
        # batch 3 and outproj(1,2) in the drain, where their rv tiles are
        # long since ready no matter how far the LDW is hoisted.
        OP_SCHED = {(3, i): [(0, i)] for i in range(8)}
        DRAIN_OP = [(ob, ci) for ob in (1, 2) for ci in range(8)]

        pend_norm = []

        for b in range(B):
            kt_sb = kt_d[b]
            qt_sb = qt_d[b]

            for i5 in range(NI):
                slot = b * NI + i5
                fill = list(fills.pop(slot, ()))
                for ob, cc in OP_SCHED.get((b, i5), []):
                    fill.append(lambda ob=ob, cc=cc: emit_outproj_chunk(ob, cc))

                isl = slice(i5 * 512, (i5 + 1) * 512)
                na = noutp.tile([65, 512], F32, name=f"na_{b}_{i5}", tag="nout")
                nb = noutp.tile([65, 512], F32, name=f"nb_{b}_{i5}", tag="nout")

                def emit_scores(jc):
                    sc = scp.tile([128, 1024], F32, name=f"sc_{b}_{i5}_{jc}", tag="sc")
                    jsl = slice(jc * 128, (jc + 1) * 128)
                    # scoresT for both heads, row-tiled (K=64 each, concurrent)
                    nc.tensor.matmul(
                        sc[:, 0:512],
                        lhsT=kt_sb[0:64, jsl],
                        rhs=qt_sb[0:64, isl],
                        start=True, stop=True,
                    )
                    nc.tensor.matmul(
                        sc[:, 512:1024],
                        lhsT=kt_sb[64:128, jsl],
                        rhs=qt_sb[64:128, isl],
                        start=True, stop=True,
                    )
                    e_t = ep.tile([128, 1024], BF16, name=f"e_{b}_{i5}_{jc}", tag="e")
                    nc.scalar.activation(e_t[:, :], sc[:, :], Exp)
                    return e_t

                # software-pipelined over jc: scores(jc+1) and filler work run
                # while ACT computes exp(jc)
                e_cur = emit_scores(0)
                nfill = len(fill)
                for jc in range(JC):
                    f0 = jc * nfill // JC
                    f1 = (jc + 1) * nfill // JC
                    for f in fill[f0:f1]:
                        f()
                    e_next = emit_scores(jc + 1) if jc + 1 < JC else None
                    v_t = v_tiles[(b, jc)]
                    nc.tensor.matmul(
                        na[:, :],
                        lhsT=v_t[:, 0:65],
                        rhs=e_cur[:, 0:512],
                        start=(jc == 0),
                        stop=(jc == JC - 1),
                    )
                    nc.tensor.matmul(
                        nb[:, :],
                        lhsT=v_t[:, 65:130],
                        rhs=e_cur[:, 512:1024],
                        start=(jc == 0),
                        stop=(jc == JC - 1),
                    )
                    e_cur = e_next

                # evacuate nout psum, normalize by the sums row, stage fp8 att
                # tiles; one send DMA per (i5-pair, head)
                if i5 % 2 == 0:
                    att_d[0] = attp.tile([64, 1024], BF16, name=f"attA_{b}_{i5}", tag="att")
                    att_d[1] = attp.tile([64, 1024], BF16, name=f"attB_{b}_{i5}", tag="att")
                hsl = slice((i5 % 2) * 512, (i5 % 2) * 512 + 512)
                # emit the PREVIOUS i5's deferred broadcast+multiply first:
                # by now its rb ride has landed, so the gpsimd queue never
                # stalls on the DRAM round-trip
                for fn in pend_norm:
                    fn()
                pend_norm = []
                for h, nres in ((0, na), (1, nb)):
                    att_u = attup.tile([65, 512], BF16, name=f"au_{b}_{i5}_{h}", tag="au")
                    nc.vector.tensor_copy(att_u[:, :], nres[:, :])
                    rec = recp.tile([65, 512], F32, name=f"rec_{b}_{i5}_{h}", tag="rec")
                    nc.vector.reciprocal_approx_fast(out=rec[:, :], in_=nres[:, :])
                    recb = recbp.tile([65, 512], BF16, name=f"rcb_{b}_{i5}_{h}", tag="rcb")
                    nc.vector.tensor_copy(recb[:, :], rec[:, :])
                    rb = rbp.tile([1, 512], BF16, name=f"rb_{b}_{i5}_{h}", tag="rb")
                    nc.gpsimd.dma_start(out=rb[:, :], in_=recb[64:65, :])

                    def norm_tail(h=h, rb=rb, att_u=att_u, att=att_d[h],
                                  hsl=hsl, b=b, i5=i5):
                        bc = bcp.tile([64, 512], BF16, name=f"bc_{b}_{i5}_{h}", tag="bc")
                        nc.gpsimd.dma_start(
                            out=bc[:, :], in_=rb[0:1, :].to_broadcast([64, 512])
                        )
                        nc.gpsimd.tensor_mul(att[:, hsl], att_u[0:64, :], bc[:, :])
                        if i5 % 2 == 1:
                            if b < B - 1:
                                nc.gpsimd.dma_start(
                                    out=send[b][i5 - 1:i5 + 1, h * 64:(h + 1) * 64, :]
                                        .rearrange("d p c -> p d c"),
                                    in_=att[:, :].rearrange("p (d c) -> p d c", d=2),
                                )
                            else:
                                for hf in range(2):
                                    nc.gpsimd.dma_start(
                                        out=send[b][hf][i5 - 1:i5 + 1,
                                                        h * 64:(h + 1) * 64, :]
                                            .rearrange("d p c -> p d c"),
                                        in_=att[:, :].rearrange(
                                            "p (d c) -> p d c", d=2
                                        )[:, :, hf * 256:(hf + 1) * 256],
                                    )
                    pend_norm.append(norm_tail)
                if i5 == NI - 1:
                    # batch boundary: flush immediately so the A2A can trigger
                    for fn in pend_norm:
                        fn()
                    pend_norm = []

            # ---- AllToAll for this batch: head-shard -> seq-shard. The
            # gather is emitted on the same gpsimd queue right after the
            # collective, so its trigger never waits (an A2A-gated gather
            # head-blocked an in-order queue for ~40us in v5).
            if b < B - 1:
                nc.gpsimd.collective_compute(
                    "AllToAll",
                    Alu.bypass,
                    replica_groups=[list(range(NCORES))],
                    ins=[send[b][:, :, :].opt()],
                    outs=[recv[b][:, :, :].opt()],
                )
                emit_rv_gather(b)
            else:
                for hf in range(2):
                    nc.gpsimd.collective_compute(
                        "AllToAll",
                        Alu.bypass,
                        replica_groups=[list(range(NCORES))],
                        ins=[send[b][hf][:, :, :].opt()],
                        outs=[recv[b][hf][:, :, :].opt()],
                    )
                    emit_rv_gather(b, hf)

        # ---- drain: outproj(2) chunks 3..7 cover A2A(3) trigger+wire, then
        # batch 3's out-projection (chunks 0..3 need only the first A2A half)
        for ob, ci in DRAIN_OP:
            emit_outproj_chunk(ob, ci)
        for chunk in range(8):
            emit_outproj_chunk(B - 1, chunk)


def prep_in_maps(x, y, Wq, bq, Wk, bk, Wv, bv, Wo, bo):
    f8 = ml_dtypes.float8_e4m3fn
    bf = ml_dtypes.bfloat16
    x = np.asarray(x, np.float32)
    y = np.asarray(y, np.float32)
    xt = np.ascontiguousarray(x.transpose(0, 2, 1)).reshape(B, KC, 128, SQ).astype(f8)
    yt = np.ascontiguousarray(y.transpose(0, 2, 1)).reshape(B, FC, 128, SK).astype(bf)
    wo = np.ascontiguousarray(np.asarray(Wo, np.float32).reshape(KC, 128, D)).astype(bf)
    bob = np.ascontiguousarray(
        np.broadcast_to(np.asarray(bo, np.float32)[None, :], (128, D))
    )
    in_maps = []
    for c in range(NCORES):
        cs = slice(c * 128, (c + 1) * 128)
        in_maps.append({
            "xt": xt,
            "yt": yt,
            # wq scaled x64 into fp8e4's normal range; bq pre-scaled to match
            # (the kernel multiplies the q psum by 0.125/64)
            "wq": np.ascontiguousarray(np.asarray(Wq, np.float32)[:, cs].reshape(KC, 128, 128) * 64.0).astype(f8),
            "wk": np.ascontiguousarray(np.asarray(Wk, np.float32)[:, cs].reshape(FC, 128, 128)).astype(bf),
            "wv": np.ascontiguousarray(np.asarray(Wv, np.float32)[:, cs].reshape(FC, 128, 128)).astype(bf),
            "wo": wo,
            "bq": np.ascontiguousarray(np.asarray(bq, np.float32)[cs].reshape(128, 1) * 64.0),
            "bk": np.ascontiguousarray(np.asarray(bk, np.float32)[cs].reshape(128, 1)),
            "bvb": np.ascontiguousarray(
                np.broadcast_to(np.asarray(bv, np.float32)[cs][None, :], (128, 128))
            ),
            "bob": bob,
        })
    return in_maps


_NC_CACHE = None


def get_nc():
    global _NC_CACHE
    if _NC_CACHE is None:
        _NC_CACHE = build_nc()
    return _NC_CACHE


def run(in_maps, **kwargs):
    nc = get_nc()
    return bass_utils.run_bass_kernel_spmd(
        nc, in_maps, core_ids=list(range(NCORES)), **kwargs
    )


def gather(results):
    full = np.empty((B, SQ, D), np.float32)
    for c in range(NCORES):
        full[:, c * SQL:(c + 1) * SQL, :] = results[c]["out"]
    return full


def kernel(**inputs):
    in_maps = prep_in_maps(**inputs)
    res = run(in_maps)
    return gather(res.results)


if __name__ == "__main__":
    nc = build_nc()
    print("build OK")
